# revision 16
# baseline (speedup 1.0000x reference)
"""Discounted cumulative return (reverse-time linear recurrence) on 8 TRN2 cores.

    c_t = r_t + gamma * (1 - terminal_t) * c_{t+1},  c_T = 0

v3: the DVE tensor_tensor_scan runs at ~2.2 cycles/element (per-element
feedback bubble), so the scan itself was co-bottleneck with DMA in the
baseline. Two levers:

1. 16-bit I/O everywhere: rewards as fp16, output stored as fp16 and
   upcast on the host; terminal masks as uint8 expanded on the scalar
   engine (fp16 gamma would bias the product, so the scan's a-operand is
   f32 {0, gamma^2}; the scan keeps fp32 internal state).

2. Radix-2 pair decimation (host-side): with a_k = gamma*m_k,
   m_k = 1-terminal_k, the recurrence over pairs is
       c_{2i+1} = (gamma^2 M_i) c_{2i-1} + B_i,
       M_i = m_{2i} m_{2i+1},  B_i = gamma m_{2i+1} b_{2i} + b_{2i+1}
   The host precomputes M (uint8) and B (fp16); the device scans only
   T/2 elements (odd outputs), then reconstructs evens with two 2x-mode
   tensor_tensor ops: c_{2i} = (gamma m_{2i}) c_{2i-1} + b_{2i}.
   Host sends the same total bytes as undecimated (3 bytes per original
   element in, 2 out) but DVE time drops ~40%.

Layout: scan (reversed-time) order, 8 cores x 128 partitions = 1024 rows,
F=16384 elements (8192 pairs) per row + H=768-element (384-pair) halo.
The odd-chain scan writes into co_full at +1 offset so the even
reconstruction reads an aligned, already-shifted slice; stripes chain via
initial = co_full[:, i0:i0+1].
"""
import sys

sys.path.insert(0, "/opt/trn_rl_repo")
from contextlib import ExitStack

import numpy as np

import concourse.bass as bass  # noqa: F401  (engine namespaces live on nc)
import concourse.tile as tile
from concourse import bacc, mybir
from concourse.bass_utils import run_bass_kernel_spmd

T = 16777216
M = 8                  # cores
L = T // M             # 2097152 elements per core
P = 128                # partitions
F = 16384              # elements per row
H = 768                # halo elements per row
R = F + H              # loaded row length (17152)
NP = R // 2            # pairs per row (8576)
HP = H // 2            # halo pairs (384)
FP = F // 2            # main pairs (8192)
SP = 2048              # scan stripe width in pairs (FP % SP == 0)
GAMMA = 0.99


def build_nc(p=P, gamma=GAMMA):
    g2 = gamma * gamma
    nc = bacc.Bacc("TRN2", debug=False, num_devices=M)
    B_in = nc.dram_tensor("B", [p, NP], mybir.dt.float16, kind="ExternalInput")
    M_in = nc.dram_tensor("Mm", [p, NP], mybir.dt.uint8, kind="ExternalInput")
    be_in = nc.dram_tensor("be", [p, FP], mybir.dt.float16, kind="ExternalInput")
    me_in = nc.dram_tensor("me", [p, FP], mybir.dt.uint8, kind="ExternalInput")
    yo_out = nc.dram_tensor("yo", [p, FP], mybir.dt.float16, kind="ExternalOutput")
    ye_out = nc.dram_tensor("ye", [p, FP], mybir.dt.float16, kind="ExternalOutput")

    # scan stripes in pairs: halo merged into a short ramp-up stripe
    scan_stripes = [(0, HP + 512), (896, 1536), (2432, 2048),
                    (4480, 2048), (6528, 2048)]

    with tile.TileContext(nc) as tc, ExitStack() as ctx:
        full = ctx.enter_context(tc.tile_pool(name="full", bufs=1))
        apool = ctx.enter_context(tc.tile_pool(name="a", bufs=3))
        ppool = ctx.enter_context(tc.tile_pool(name="pe", bufs=3))
        tpool = ctx.enter_context(tc.tile_pool(name="tmp", bufs=3))

        Bt = full.tile([p, NP], mybir.dt.float16, tag="B")
        Mt = full.tile([p, NP], mybir.dt.uint8, tag="M")
        bet = full.tile([p, FP], mybir.dt.float16, tag="be")
        met = full.tile([p, FP], mybir.dt.uint8, tag="me")
        cot = full.tile([p, NP + 1], mybir.dt.float16, tag="co")

        # All loads on the sync HWDGE ring in scan-critical order; the
        # recon-only streams (me/be) slot in after the third stripe's B/M.
        def ld(dst, src, c0, w):
            nc.sync.dma_start(dst[:, c0 : c0 + w], src[:, c0 : c0 + w])

        # me/be are split into right-sized chunks placed just behind the
        # B/M they must not delay: each recon's inputs land ~2us before
        # its scan finishes, without pushing any scan-critical chunk late.
        for c0, w in scan_stripes[:2]:
            ld(Mt, M_in, c0, w)
            ld(Bt, B_in, c0, w)
        ld(met, me_in, 0, 512)
        ld(bet, be_in, 0, 512)
        ld(Mt, M_in, *scan_stripes[2])
        ld(Bt, B_in, *scan_stripes[2])
        ld(met, me_in, 512, 1536)
        ld(bet, be_in, 512, 1536)
        ld(Mt, M_in, *scan_stripes[3])
        ld(Bt, B_in, *scan_stripes[3])
        ld(met, me_in, 2048, 3072)
        ld(bet, be_in, 2048, 3072)
        ld(Mt, M_in, *scan_stripes[4])
        ld(Bt, B_in, *scan_stripes[4])
        ld(met, me_in, 5120, 3072)
        ld(bet, be_in, 5120, 3072)

        def emit_scan(c0, w):
            ta = apool.tile([p, w], mybir.dt.float32, tag="a")
            # A = gamma^2 * M  (f32: unbiased gamma)
            nc.scalar.activation(
                ta[:], Mt[:, c0 : c0 + w], mybir.ActivationFunctionType.Copy,
                bias=0.0, scale=g2,
            )
            # odd-position chain: co[i] = A_i co[i-1] + B_i, written at +1
            init = 0.0 if c0 == 0 else cot[:, c0 : c0 + 1]
            nc.vector.tensor_tensor_scan(
                cot[:, c0 + 1 : c0 + w + 1], ta[:], Bt[:, c0 : c0 + w],
                init,
                op0=mybir.AluOpType.mult, op1=mybir.AluOpType.add,
            )

        def emit_recon(c0, w):
            r0 = max(c0, HP)          # first main pair of this stripe
            j0 = r0 - HP              # main-pair index
            rw = c0 + w - r0          # recon width
            tp = ppool.tile([p, rw], mybir.dt.float16, tag="pe")
            # p_e = gamma * m_e   (single factor: fp16 gamma bias is ~2e-4)
            nc.scalar.activation(
                tp[:], met[:, j0 : j0 + rw], mybir.ActivationFunctionType.Copy,
                bias=0.0, scale=gamma,
            )
            tm = tpool.tile([p, rw], mybir.dt.float16, tag="tmp")
            # evens: c_{2i} = p_e * c_{2i-1} + b_e  (both ops 2x_1p)
            nc.vector.tensor_tensor(
                tm[:], tp[:], cot[:, r0 : r0 + rw], op=mybir.AluOpType.mult
            )
            nc.vector.tensor_tensor(
                bet[:, j0 : j0 + rw], tm[:], bet[:, j0 : j0 + rw],
                op=mybir.AluOpType.add,
            )
            # stores split across the scalar HWDGE and gpsimd SWDGE rings
            nc.scalar.dma_start(
                yo_out[:, j0 : j0 + rw], cot[:, r0 + 1 : r0 + rw + 1]
            )
            nc.gpsimd.dma_start(ye_out[:, j0 : j0 + rw], bet[:, j0 : j0 + rw])

        # DVE FIFO: keep the scan chain a step ahead of the recons so a
        # late me/be load can never stall the next scan.
        emit_scan(*scan_stripes[0])
        emit_scan(*scan_stripes[1])
        emit_recon(*scan_stripes[0])
        emit_scan(*scan_stripes[2])
        emit_recon(*scan_stripes[1])
        emit_scan(*scan_stripes[3])
        emit_recon(*scan_stripes[2])
        emit_scan(*scan_stripes[4])
        emit_recon(*scan_stripes[3])
        emit_recon(*scan_stripes[4])
    nc.finalize()
    return nc


def shard_inputs(terminal, reward, t=T, m=M, p=P, f=F, h=H, gamma=GAMMA):
    """Per-core pair-decimated tiles; rows and columns in scan order."""
    l = p * f
    r = f + h
    term_pad = np.concatenate(
        [np.asarray(terminal).astype(np.uint8), np.ones(h, np.uint8)])
    rew_pad = np.concatenate(
        [np.asarray(reward).astype(np.float32), np.zeros(h, np.float32)])
    tw = np.lib.stride_tricks.sliding_window_view(term_pad, r)
    rw = np.lib.stride_tricks.sliding_window_view(rew_pad, r)
    in_maps = []
    for mm in range(m):
        base = t - (mm + 1) * l
        rows = base + (p - 1 - np.arange(p)) * f
        ms = 1 - tw[rows][:, ::-1]          # m = 1 - terminal, scan order
        bs = rw[rows][:, ::-1]              # rewards, scan order
        m_e, m_o = ms[:, 0::2], ms[:, 1::2]
        b_e, b_o = bs[:, 0::2], bs[:, 1::2]
        in_maps.append({
            "B": (gamma * m_o * b_e + b_o).astype(np.float16),
            "Mm": np.ascontiguousarray(m_e * m_o),
            "be": np.ascontiguousarray(b_e[:, h // 2:]).astype(np.float16),
            "me": np.ascontiguousarray(m_e[:, h // 2:]),
        })
    return in_maps


def unshard_output(results, t=T, m=M, p=P, f=F):
    l = p * f
    full = np.empty(t, np.float32)
    row = np.empty((p, f), np.float32)
    for mm in range(m):
        yo = np.asarray(results[mm]["yo"])
        ye = np.asarray(results[mm]["ye"])
        base = t - (mm + 1) * l
        row[:, 0::2] = ye
        row[:, 1::2] = yo
        full[base : base + l] = row.reshape(l)[::-1]
    return full


_NC = None


def kernel(terminal, reward):
    global _NC
    if _NC is None:
        _NC = build_nc()
    in_maps = shard_inputs(terminal, reward)
    res = run_bass_kernel_spmd(_NC, in_maps, list(range(M)))
    return unshard_output(res.results)


# revision 17
# speedup vs baseline: 1.1562x; 1.1562x over previous
"""Discounted cumulative return (reverse-time linear recurrence) on 8 TRN2 cores.

    c_t = r_t + gamma * (1 - terminal_t) * c_{t+1},  c_T = 0

v3: the DVE tensor_tensor_scan runs at ~2.2 cycles/element (per-element
feedback bubble), so the scan itself was co-bottleneck with DMA in the
baseline. Two levers:

1. 16-bit I/O everywhere: rewards as fp16, output stored as fp16 and
   upcast on the host; terminal masks as uint8 expanded on the scalar
   engine (fp16 gamma would bias the product, so the scan's a-operand is
   f32 {0, gamma^2}; the scan keeps fp32 internal state).

2. Radix-2 pair decimation (host-side): with a_k = gamma*m_k,
   m_k = 1-terminal_k, the recurrence over pairs is
       c_{2i+1} = (gamma^2 M_i) c_{2i-1} + B_i,
       M_i = m_{2i} m_{2i+1},  B_i = gamma m_{2i+1} b_{2i} + b_{2i+1}
   The host precomputes M (uint8) and B (fp16); the device scans only
   T/2 elements (odd outputs), then reconstructs evens with two 2x-mode
   tensor_tensor ops: c_{2i} = (gamma m_{2i}) c_{2i-1} + b_{2i}.
   Host sends the same total bytes as undecimated (3 bytes per original
   element in, 2 out) but DVE time drops ~40%.

Layout: scan (reversed-time) order, 8 cores x 128 partitions = 1024 rows,
F=16384 elements (8192 pairs) per row + H=768-element (384-pair) halo.
The odd-chain scan writes into co_full at +1 offset so the even
reconstruction reads an aligned, already-shifted slice; stripes chain via
initial = co_full[:, i0:i0+1].
"""
import sys

sys.path.insert(0, "/opt/trn_rl_repo")
from contextlib import ExitStack

import numpy as np

import concourse.bass as bass  # noqa: F401  (engine namespaces live on nc)
import concourse.tile as tile
from concourse import bacc, mybir
from concourse.bass_utils import run_bass_kernel_spmd

T = 16777216
M = 8                  # cores
L = T // M             # 2097152 elements per core
P = 128                # partitions
F = 16384              # elements per row
H = 768                # halo elements per row
R = F + H              # loaded row length (17152)
NP = R // 2            # pairs per row (8576)
HP = H // 2            # halo pairs (384)
FP = F // 2            # main pairs (8192)
SP = 2048              # scan stripe width in pairs (FP % SP == 0)
GAMMA = 0.99


def build_nc(p=P, gamma=GAMMA):
    g2 = gamma * gamma
    nc = bacc.Bacc("TRN2", debug=False, num_devices=M)
    B_in = nc.dram_tensor("B", [p, NP], mybir.dt.float16, kind="ExternalInput")
    M_in = nc.dram_tensor("Mm", [p, NP], mybir.dt.uint8, kind="ExternalInput")
    be_in = nc.dram_tensor("be", [p, FP], mybir.dt.float16, kind="ExternalInput")
    me_in = nc.dram_tensor("me", [p, FP], mybir.dt.uint8, kind="ExternalInput")
    yo_out = nc.dram_tensor("yo", [p, FP], mybir.dt.float16, kind="ExternalOutput")
    ye_out = nc.dram_tensor("ye", [p, FP], mybir.dt.float16, kind="ExternalOutput")

    # scan stripes in pairs: halo merged into a short ramp-up stripe, short
    # last stripe so the store tail is small
    scan_stripes = [(0, HP + 1024), (1408, 2048), (3456, 2048),
                    (5504, 2048), (7552, 1024)]

    with tile.TileContext(nc) as tc, ExitStack() as ctx:
        full = ctx.enter_context(tc.tile_pool(name="full", bufs=1))
        apool = ctx.enter_context(tc.tile_pool(name="a", bufs=3))
        ppool = ctx.enter_context(tc.tile_pool(name="pe", bufs=3))
        tpool = ctx.enter_context(tc.tile_pool(name="tmp", bufs=3))

        Bt = full.tile([p, NP], mybir.dt.float16, tag="B")
        Mt = full.tile([p, NP], mybir.dt.uint8, tag="M")
        bet = full.tile([p, FP], mybir.dt.float16, tag="be")
        met = full.tile([p, FP], mybir.dt.uint8, tag="me")
        cot = full.tile([p, NP + 1], mybir.dt.float16, tag="co")

        # All loads on the sync HWDGE ring in scan-critical order; the
        # recon-only streams (me/be) slot in after the third stripe's B/M.
        def ld(dst, src, c0, w):
            nc.sync.dma_start(dst[:, c0 : c0 + w], src[:, c0 : c0 + w])

        # me/be are split into right-sized chunks placed just behind the
        # B/M they must not delay: each recon's inputs land ~2us before
        # its scan finishes, without pushing any scan-critical chunk late.
        for c0, w in scan_stripes[:2]:
            ld(Mt, M_in, c0, w)
            ld(Bt, B_in, c0, w)
        ld(met, me_in, 0, 1024)
        ld(bet, be_in, 0, 1024)
        ld(Mt, M_in, *scan_stripes[2])
        ld(Bt, B_in, *scan_stripes[2])
        ld(met, me_in, 1024, 2048)
        ld(bet, be_in, 1024, 2048)
        ld(Mt, M_in, *scan_stripes[3])
        ld(Bt, B_in, *scan_stripes[3])
        ld(met, me_in, 3072, 5120)
        ld(bet, be_in, 3072, 5120)
        ld(Mt, M_in, *scan_stripes[4])
        ld(Bt, B_in, *scan_stripes[4])

        def emit_scan(c0, w):
            ta = apool.tile([p, w], mybir.dt.float32, tag="a")
            # A = gamma^2 * M  (f32: unbiased gamma)
            nc.scalar.activation(
                ta[:], Mt[:, c0 : c0 + w], mybir.ActivationFunctionType.Copy,
                bias=0.0, scale=g2,
            )
            # odd-position chain: co[i] = A_i co[i-1] + B_i, written at +1
            init = 0.0 if c0 == 0 else cot[:, c0 : c0 + 1]
            nc.vector.tensor_tensor_scan(
                cot[:, c0 + 1 : c0 + w + 1], ta[:], Bt[:, c0 : c0 + w],
                init,
                op0=mybir.AluOpType.mult, op1=mybir.AluOpType.add,
            )

        def emit_recon(c0, w):
            r0 = max(c0, HP)          # first main pair of this stripe
            j0 = r0 - HP              # main-pair index
            rw = c0 + w - r0          # recon width
            tp = ppool.tile([p, rw], mybir.dt.float16, tag="pe")
            # p_e = gamma * m_e   (single factor: fp16 gamma bias is ~2e-4)
            nc.scalar.activation(
                tp[:], met[:, j0 : j0 + rw], mybir.ActivationFunctionType.Copy,
                bias=0.0, scale=gamma,
            )
            tm = tpool.tile([p, rw], mybir.dt.float16, tag="tmp")
            # evens: c_{2i} = p_e * c_{2i-1} + b_e  (both ops 2x_1p)
            nc.vector.tensor_tensor(
                tm[:], tp[:], cot[:, r0 : r0 + rw], op=mybir.AluOpType.mult
            )
            nc.vector.tensor_tensor(
                bet[:, j0 : j0 + rw], tm[:], bet[:, j0 : j0 + rw],
                op=mybir.AluOpType.add,
            )
            # stores split across the scalar HWDGE and gpsimd SWDGE rings
            nc.scalar.dma_start(
                yo_out[:, j0 : j0 + rw], cot[:, r0 + 1 : r0 + rw + 1]
            )
            nc.gpsimd.dma_start(ye_out[:, j0 : j0 + rw], bet[:, j0 : j0 + rw])

        # DVE FIFO: keep the scan chain a step ahead of the recons so a
        # late me/be load can never stall the next scan.
        emit_scan(*scan_stripes[0])
        emit_scan(*scan_stripes[1])
        emit_recon(*scan_stripes[0])
        emit_scan(*scan_stripes[2])
        emit_recon(*scan_stripes[1])
        emit_scan(*scan_stripes[3])
        emit_recon(*scan_stripes[2])
        emit_scan(*scan_stripes[4])
        emit_recon(*scan_stripes[3])
        emit_recon(*scan_stripes[4])
    nc.finalize()
    return nc


def shard_inputs(terminal, reward, t=T, m=M, p=P, f=F, h=H, gamma=GAMMA):
    """Per-core pair-decimated tiles; rows and columns in scan order."""
    l = p * f
    r = f + h
    term_pad = np.concatenate(
        [np.asarray(terminal).astype(np.uint8), np.ones(h, np.uint8)])
    rew_pad = np.concatenate(
        [np.asarray(reward).astype(np.float32), np.zeros(h, np.float32)])
    tw = np.lib.stride_tricks.sliding_window_view(term_pad, r)
    rw = np.lib.stride_tricks.sliding_window_view(rew_pad, r)
    in_maps = []
    for mm in range(m):
        base = t - (mm + 1) * l
        rows = base + (p - 1 - np.arange(p)) * f
        ms = 1 - tw[rows][:, ::-1]          # m = 1 - terminal, scan order
        bs = rw[rows][:, ::-1]              # rewards, scan order
        m_e, m_o = ms[:, 0::2], ms[:, 1::2]
        b_e, b_o = bs[:, 0::2], bs[:, 1::2]
        in_maps.append({
            "B": (gamma * m_o * b_e + b_o).astype(np.float16),
            "Mm": np.ascontiguousarray(m_e * m_o),
            "be": np.ascontiguousarray(b_e[:, h // 2:]).astype(np.float16),
            "me": np.ascontiguousarray(m_e[:, h // 2:]),
        })
    return in_maps


def unshard_output(results, t=T, m=M, p=P, f=F):
    l = p * f
    full = np.empty(t, np.float32)
    row = np.empty((p, f), np.float32)
    for mm in range(m):
        yo = np.asarray(results[mm]["yo"])
        ye = np.asarray(results[mm]["ye"])
        base = t - (mm + 1) * l
        row[:, 0::2] = ye
        row[:, 1::2] = yo
        full[base : base + l] = row.reshape(l)[::-1]
    return full


_NC = None


def kernel(terminal, reward):
    global _NC
    if _NC is None:
        _NC = build_nc()
    in_maps = shard_inputs(terminal, reward)
    res = run_bass_kernel_spmd(_NC, in_maps, list(range(M)))
    return unshard_output(res.results)
